# revision 7
# baseline (speedup 1.0000x reference)
"""Trainium2 Bass kernel for nn_Attention (Bahdanau-style additive attention).

Computation (reference):
    enc = encoder_outputs.transpose(1, 0, 2)            # [B, S, 2H]
    e_proj = enc @ w_e.T                                # [B, S, H]
    energy = tanh(h_proj[:, None, :] + e_proj + b)      # [B, S, H]
    att = energy @ v_w                                  # [B, S]
    out = softmax(att, axis=1)

Strategy: data-parallel over batch, 4 batch rows per core on 8 cores.
Per core, the big matmul (S x 2H) @ (2H x H) runs in bf16 on the PE:
  - encoder slice is DMA-transposed (xbar) from DRAM bf16 [S, 2H] into
    SBUF [128, 16, SG] so the contraction dim (e) lands on partitions;
    all 4 s-groups of a batch row stay resident so each weight chunk
    (the PE stationary operand) is reused for 4 matmuls and LDWEIGHTS
    stays hidden
  - psum[h_chunk(128), s(512)] accumulates over 16 e-chunks
  - ACT applies tanh with the per-partition bias c_b = h_proj + attn_b
    (h_proj is tiny: computed on host in fp32)
  - the v-dot reduction over h runs as an M=1 matmul accumulated over
    8 h-chunks; the 4 s-groups of a batch row share one PSUM bank at
    partitions {0,32,64,96} via tile_position
Softmax over S (tiny, [32, 2048]) runs on host in fp32.
"""

import sys

try:
    import concourse.bass as bass  # noqa: F401
except ImportError:
    sys.path.insert(0, "/opt/trn_rl_repo")

import numpy as np
import ml_dtypes

import concourse.bacc as bacc
import concourse.mybir as mybir
import concourse.tile as tile
from concourse.bass_utils import run_bass_kernel_spmd

HID = 1024
BATCH = 32
SRC_LEN = 2048

N_CORES = 8
B_LOC = BATCH // N_CORES      # 4
E = 2 * HID                   # 2048
SG = 512                      # matmul moving free dim (s per group)
N_SG = SRC_LEN // SG          # 4
N_EC = E // 128               # 16 e-chunks
N_HC = HID // 128             # 8 h-chunks

f32 = mybir.dt.float32
bf16 = mybir.dt.bfloat16

_NC_CACHE = {}


def _build2():
    """Clean version: drains placed right after the deferred v-dots."""
    nc = bacc.Bacc(
        "TRN2", target_bir_lowering=False, debug=False, num_devices=N_CORES
    )
    enc = nc.declare_dram_parameter("enc", [B_LOC, SRC_LEN, E], bf16, isOutput=False)
    wT = nc.declare_dram_parameter("wT", [E, HID], bf16, isOutput=False)
    cb = nc.declare_dram_parameter("cb", [128, B_LOC * N_HC], f32, isOutput=False)
    vT = nc.declare_dram_parameter("vT", [128, N_HC], bf16, isOutput=False)
    att = nc.declare_dram_parameter("att", [N_SG, B_LOC, SG], f32, isOutput=True)

    with tile.TileContext(nc) as tc:
        with (
            tc.tile_pool(name="const", bufs=1) as const_pool,
            tc.tile_pool(name="encT", bufs=2 * N_SG) as encT_pool,
            tc.tile_pool(name="energy", bufs=8) as en_pool,
            tc.tile_pool(name="attsb", bufs=1) as att_pool,
            tc.tile_pool(name="psum", bufs=7, space="PSUM") as psum_pool,
            tc.tile_pool(name="attps", bufs=1, space="PSUM") as attps_pool,
        ):
            # constants go on the ACT HWDGE queue so they don't serialize
            # behind the encoder transposes on the sync queue; the weight
            # load is split per e-chunk so the first matmul only waits for
            # chunk 0
            w_sb = const_pool.tile([128, N_EC, HID], bf16)
            wT3 = wT.rearrange("(c p) h -> p c h", p=128)
            for c in range(N_EC):
                nc.scalar.dma_start(w_sb[:, c:c + 1, :], wT3[:, c:c + 1, :])
            cb_sb = const_pool.tile([128, B_LOC * N_HC], f32)
            nc.scalar.dma_start(cb_sb[:], cb[:])
            vT_sb = const_pool.tile([128, N_HC], bf16)
            nc.scalar.dma_start(vT_sb[:], vT[:])
            att_all = att_pool.tile([128, B_LOC * SG], f32)

            warm = const_pool.tile([128, 1], f32)
            nc.scalar.activation(
                warm[:], cb_sb[:, 0:1], mybir.ActivationFunctionType.Tanh
            )

            pending = []      # (att_ps, energy, hc, sg)
            done_banks = []   # (att_ps, b) fully accumulated

            def emit_vdots():
                while pending:
                    p_att, p_en, p_hc, p_sg = pending.pop(0)
                    nc.tensor.matmul(
                        p_att[32 * p_sg:32 * p_sg + 1, :],
                        lhsT=vT_sb[:, p_hc:p_hc + 1],
                        rhs=p_en[:],
                        start=(p_hc == 0),
                        stop=(p_hc == N_HC - 1),
                        tile_position=(0, 32 * p_sg),
                    )

            def drain_banks():
                while done_banks:
                    d_ps, d_b = done_banks.pop(0)
                    nc.vector.tensor_copy(
                        att_all[:, d_b * SG:(d_b + 1) * SG], d_ps[:]
                    )

            HALF = N_EC // 2
            for b in range(B_LOC):
                encTs = []
                for sg in range(N_SG):
                    # two half-tile transposes per s-group: finer dependency
                    # granularity so the first matmuls start ~3.5us in
                    encT = encT_pool.tile([128, N_EC, SG], bf16)
                    for half in range(2):
                        nc.sync.dma_start(
                            out=encT[:, half * HALF:(half + 1) * HALF, :],
                            in_=enc[
                                b,
                                sg * SG:(sg + 1) * SG,
                                half * HALF * 128:(half + 1) * HALF * 128,
                            ],
                            transpose=True,
                        )
                    encTs.append(encT)
                att_ps = attps_pool.tile([128, SG], f32)
                for hc in range(N_HC):
                    pss = [
                        psum_pool.tile([128, SG], f32, tag="ps", name=f"ps_{b}_{hc}_{i}")
                        for i in range(N_SG)
                    ]
                    if b == 0 and hc == 0:
                        # startup: consume each s-group tile as its transpose
                        # lands instead of round-robining across all four
                        for sg in range(N_SG):
                            for c in range(N_EC):
                                nc.tensor.matmul(
                                    pss[sg][:],
                                    lhsT=w_sb[:, c, hc * 128:(hc + 1) * 128],
                                    rhs=encTs[sg][:, c, :],
                                    start=(c == 0),
                                    stop=(c == N_EC - 1),
                                )
                    else:
                        for c in range(N_EC):
                            for sg in range(N_SG):
                                nc.tensor.matmul(
                                    pss[sg][:],
                                    lhsT=w_sb[:, c, hc * 128:(hc + 1) * 128],
                                    rhs=encTs[sg][:, c, :],
                                    start=(c == 0),
                                    stop=(c == N_EC - 1),
                                )
                            if c == 1:
                                emit_vdots()
                                drain_banks()
                    for sg in range(N_SG):
                        energy = en_pool.tile([128, SG], bf16)
                        nc.scalar.activation(
                            energy[:], pss[sg][:],
                            mybir.ActivationFunctionType.Tanh,
                            bias=cb_sb[:, b * N_HC + hc:b * N_HC + hc + 1],
                            scale=1.0,
                        )
                        pending.append((att_ps, energy, hc, sg))
                done_banks.append((att_ps, b))
            emit_vdots()
            drain_banks()
            nc.sync.dma_start(
                att[:],
                att_all.rearrange("(q t) (b s) -> q t b s", t=32, b=B_LOC)[:, 0],
            )
    nc.compile()
    return nc


def _get_nc():
    if "nc" not in _NC_CACHE:
        _NC_CACHE["nc"] = _build2()
    return _NC_CACHE["nc"]


def kernel(hidden, encoder_outputs, attn_w, attn_b, v_w, _trace=False):
    hidden = np.asarray(hidden, dtype=np.float32)
    encoder_outputs = np.asarray(encoder_outputs, dtype=np.float32)
    attn_w = np.asarray(attn_w, dtype=np.float32)
    attn_b = np.asarray(attn_b, dtype=np.float32)
    v_w = np.asarray(v_w, dtype=np.float32)

    # host-side prologue (tiny): h_proj + bias
    c_b = hidden @ attn_w[:, :HID].T + attn_b          # [B, H] fp32
    wT_bf = np.ascontiguousarray(attn_w[:, HID:].T).astype(ml_dtypes.bfloat16)
    vT_dev = np.ascontiguousarray(
        v_w.reshape(N_HC, 128).T
    ).astype(ml_dtypes.bfloat16)

    nc = _get_nc()
    in_maps = []
    for core in range(N_CORES):
        b0 = core * B_LOC
        enc_bf = np.ascontiguousarray(
            encoder_outputs[:, b0:b0 + B_LOC, :].transpose(1, 0, 2)
        ).astype(ml_dtypes.bfloat16)
        cb_dev = np.ascontiguousarray(
            c_b[b0:b0 + B_LOC]
            .reshape(B_LOC, N_HC, 128)
            .transpose(2, 0, 1)
            .reshape(128, B_LOC * N_HC)
        )
        in_maps.append({"enc": enc_bf, "wT": wT_bf, "cb": cb_dev, "vT": vT_dev})

    res = run_bass_kernel_spmd(
        nc, in_maps, core_ids=list(range(N_CORES)), trace=_trace
    )
    if _trace:
        _NC_CACHE["last_result"] = res

    att = np.concatenate(
        [
            res.results[c]["att"].transpose(1, 0, 2).reshape(B_LOC, SRC_LEN)
            for c in range(N_CORES)
        ],
        axis=0,
    )  # [B, S] logits, fp32

    # host softmax over S
    m = att.max(axis=1, keepdims=True)
    e = np.exp(att - m)
    out = e / e.sum(axis=1, keepdims=True)
    return out.astype(np.float32)


# revision 8
# speedup vs baseline: 1.0206x; 1.0206x over previous
"""Trainium2 Bass kernel for nn_Attention (Bahdanau-style additive attention).

Computation (reference):
    enc = encoder_outputs.transpose(1, 0, 2)            # [B, S, 2H]
    e_proj = enc @ w_e.T                                # [B, S, H]
    energy = tanh(h_proj[:, None, :] + e_proj + b)      # [B, S, H]
    att = energy @ v_w                                  # [B, S]
    out = softmax(att, axis=1)

Strategy: data-parallel over batch, 4 batch rows per core on 8 cores.
Per core, the big matmul (S x 2H) @ (2H x H) runs in bf16 on the PE:
  - encoder slice is DMA-transposed (xbar) from DRAM bf16 [S, 2H] into
    SBUF [128, 16, SG] so the contraction dim (e) lands on partitions;
    all 4 s-groups of a batch row stay resident so each weight chunk
    (the PE stationary operand) is reused for 4 matmuls and LDWEIGHTS
    stays hidden
  - psum[h_chunk(128), s(512)] accumulates over 16 e-chunks
  - ACT applies tanh with the per-partition bias c_b = h_proj + attn_b
    (h_proj is tiny: computed on host in fp32)
  - the v-dot reduction over h runs as an M=1 matmul accumulated over
    8 h-chunks; the 4 s-groups of a batch row share one PSUM bank at
    partitions {0,32,64,96} via tile_position
All DMAs (including xbar transposes) execute serially in emission order
(Tile serializes DMATranspose vs DMACopy transitions), so the weight
load is split by h-slice and only slice 0 gates the first matmuls; the
rest stream in behind batch row 0's transposes.
Softmax over S (tiny, [32, 2048]) runs on host in fp32.
"""

import sys

try:
    import concourse.bass as bass  # noqa: F401
except ImportError:
    sys.path.insert(0, "/opt/trn_rl_repo")

import numpy as np
import ml_dtypes

import concourse.bacc as bacc
import concourse.mybir as mybir
import concourse.tile as tile
from concourse.bass_utils import run_bass_kernel_spmd

HID = 1024
BATCH = 32
SRC_LEN = 2048

N_CORES = 8
B_LOC = BATCH // N_CORES      # 4
E = 2 * HID                   # 2048
SG = 512                      # matmul moving free dim (s per group)
N_SG = SRC_LEN // SG          # 4
N_EC = E // 128               # 16 e-chunks
N_HC = HID // 128             # 8 h-chunks

f32 = mybir.dt.float32
bf16 = mybir.dt.bfloat16

_NC_CACHE = {}


def _build():
    nc = bacc.Bacc(
        "TRN2", target_bir_lowering=False, debug=False, num_devices=N_CORES
    )
    enc = nc.declare_dram_parameter("enc", [B_LOC, SRC_LEN, E], bf16, isOutput=False)
    wT = nc.declare_dram_parameter("wT", [E, HID], bf16, isOutput=False)
    cb = nc.declare_dram_parameter("cb", [128, B_LOC * N_HC], f32, isOutput=False)
    vT = nc.declare_dram_parameter("vT", [128, N_HC], bf16, isOutput=False)
    att = nc.declare_dram_parameter("att", [N_SG, B_LOC, SG], f32, isOutput=True)

    with tile.TileContext(nc) as tc:
        with (
            tc.tile_pool(name="const", bufs=1) as const_pool,
            tc.tile_pool(name="encT", bufs=2 * N_SG) as encT_pool,
            tc.tile_pool(name="energy", bufs=8) as en_pool,
            tc.tile_pool(name="attsb", bufs=1) as att_pool,
            tc.tile_pool(name="psum", bufs=7, space="PSUM") as psum_pool,
            tc.tile_pool(name="attps", bufs=1, space="PSUM") as attps_pool,
        ):
            w_sb = const_pool.tile([128, N_EC, HID], bf16)
            wT3 = wT.rearrange("(c p) h -> p c h", p=128)
            cb_sb = const_pool.tile([128, B_LOC * N_HC], f32)
            vT_sb = const_pool.tile([128, N_HC], bf16)
            att_all = att_pool.tile([128, B_LOC * SG], f32)

            def load_w_slice(hc):
                nc.sync.dma_start(
                    w_sb[:, :, hc * 128:(hc + 1) * 128],
                    wT3[:, :, hc * 128:(hc + 1) * 128],
                )

            # h-slice 0 of the weights + the small consts: everything the
            # first matmul burst and first tanh need
            load_w_slice(0)
            nc.sync.dma_start(cb_sb[:], cb[:])
            nc.sync.dma_start(vT_sb[:], vT[:])

            # warmup tanh: pulls the ACT LUT-table-load dependency off the
            # first real tanh so no instruction needs multiple sync waits
            warm = const_pool.tile([128, 1], f32)
            nc.scalar.activation(
                warm[:], cb_sb[:, 0:1], mybir.ActivationFunctionType.Tanh
            )

            pending = []      # (att_ps, energy, hc, sg)
            done_banks = []   # (att_ps, b) fully accumulated

            def emit_vdots():
                while pending:
                    p_att, p_en, p_hc, p_sg = pending.pop(0)
                    nc.tensor.matmul(
                        p_att[32 * p_sg:32 * p_sg + 1, :],
                        lhsT=vT_sb[:, p_hc:p_hc + 1],
                        rhs=p_en[:],
                        start=(p_hc == 0),
                        stop=(p_hc == N_HC - 1),
                        tile_position=(0, 32 * p_sg),
                    )

            def drain_banks():
                while done_banks:
                    d_ps, d_b = done_banks.pop(0)
                    nc.vector.tensor_copy(
                        att_all[:, d_b * SG:(d_b + 1) * SG], d_ps[:]
                    )

            HALF = N_EC // 2

            def transpose_group(b, sg, split):
                encT = encT_pool.tile(
                    [128, N_EC, SG], bf16, tag="encT", name=f"encT_{b}_{sg}"
                )
                n_parts = 2 if split else 1
                step = N_EC // n_parts
                for part in range(n_parts):
                    nc.sync.dma_start(
                        out=encT[:, part * step:(part + 1) * step, :],
                        in_=enc[
                            b,
                            sg * SG:(sg + 1) * SG,
                            part * step * 128:(part + 1) * step * 128,
                        ],
                        transpose=True,
                    )
                return encT

            def tanh_energy(pss, b, hc, att_ps):
                for sg in range(N_SG):
                    energy = en_pool.tile(
                        [128, SG], bf16, tag="en", name=f"en_{b}_{hc}_{sg}"
                    )
                    nc.scalar.activation(
                        energy[:], pss[sg][:],
                        mybir.ActivationFunctionType.Tanh,
                        bias=cb_sb[:, b * N_HC + hc:b * N_HC + hc + 1],
                        scale=1.0,
                    )
                    pending.append((att_ps, energy, hc, sg))

            for b in range(B_LOC):
                if b == 0:
                    # batch row 0: transposes split in halves, weight slices
                    # interleaved behind them on the serial DMA chain
                    encTs = [transpose_group(0, sg, split=True) for sg in range(2)]
                    for hc in range(1, N_HC):
                        load_w_slice(hc)
                    encTs += [transpose_group(0, sg, split=True) for sg in range(2, 4)]
                else:
                    encTs = [transpose_group(b, sg, split=False) for sg in range(N_SG)]
                att_ps = attps_pool.tile(
                    [128, SG], f32, tag="attps", name=f"attps_{b}"
                )
                for hc in range(N_HC):
                    pss = [
                        psum_pool.tile([128, SG], f32, tag="ps", name=f"ps_{b}_{hc}_{i}")
                        for i in range(N_SG)
                    ]
                    if b == 0 and hc == 0:
                        # startup: consume each s-group tile as its transpose
                        # lands instead of round-robining across all four
                        for sg in range(N_SG):
                            for c in range(N_EC):
                                nc.tensor.matmul(
                                    pss[sg][:],
                                    lhsT=w_sb[:, c, hc * 128:(hc + 1) * 128],
                                    rhs=encTs[sg][:, c, :],
                                    start=(c == 0),
                                    stop=(c == N_EC - 1),
                                )
                    else:
                        for c in range(N_EC):
                            for sg in range(N_SG):
                                nc.tensor.matmul(
                                    pss[sg][:],
                                    lhsT=w_sb[:, c, hc * 128:(hc + 1) * 128],
                                    rhs=encTs[sg][:, c, :],
                                    start=(c == 0),
                                    stop=(c == N_EC - 1),
                                )
                            if c == 1:
                                emit_vdots()
                                drain_banks()
                    tanh_energy(pss, b, hc, att_ps)
                done_banks.append((att_ps, b))
            emit_vdots()
            drain_banks()
            nc.sync.dma_start(
                att[:],
                att_all.rearrange("(q t) (b s) -> q t b s", t=32, b=B_LOC)[:, 0],
            )
    nc.compile()
    return nc


def _get_nc():
    if "nc" not in _NC_CACHE:
        _NC_CACHE["nc"] = _build()
    return _NC_CACHE["nc"]


def kernel(hidden, encoder_outputs, attn_w, attn_b, v_w, _trace=False):
    hidden = np.asarray(hidden, dtype=np.float32)
    encoder_outputs = np.asarray(encoder_outputs, dtype=np.float32)
    attn_w = np.asarray(attn_w, dtype=np.float32)
    attn_b = np.asarray(attn_b, dtype=np.float32)
    v_w = np.asarray(v_w, dtype=np.float32)

    # host-side prologue (tiny): h_proj + bias
    c_b = hidden @ attn_w[:, :HID].T + attn_b          # [B, H] fp32
    wT_bf = np.ascontiguousarray(attn_w[:, HID:].T).astype(ml_dtypes.bfloat16)
    vT_dev = np.ascontiguousarray(
        v_w.reshape(N_HC, 128).T
    ).astype(ml_dtypes.bfloat16)

    nc = _get_nc()
    in_maps = []
    for core in range(N_CORES):
        b0 = core * B_LOC
        enc_bf = np.ascontiguousarray(
            encoder_outputs[:, b0:b0 + B_LOC, :].transpose(1, 0, 2)
        ).astype(ml_dtypes.bfloat16)
        cb_dev = np.ascontiguousarray(
            c_b[b0:b0 + B_LOC]
            .reshape(B_LOC, N_HC, 128)
            .transpose(2, 0, 1)
            .reshape(128, B_LOC * N_HC)
        )
        in_maps.append({"enc": enc_bf, "wT": wT_bf, "cb": cb_dev, "vT": vT_dev})

    res = run_bass_kernel_spmd(
        nc, in_maps, core_ids=list(range(N_CORES)), trace=_trace
    )
    if _trace:
        _NC_CACHE["last_result"] = res

    att = np.concatenate(
        [
            res.results[c]["att"].transpose(1, 0, 2).reshape(B_LOC, SRC_LEN)
            for c in range(N_CORES)
        ],
        axis=0,
    )  # [B, S] logits, fp32

    # host softmax over S
    m = att.max(axis=1, keepdims=True)
    e = np.exp(att - m)
    out = e / e.sum(axis=1, keepdims=True)
    return out.astype(np.float32)


# revision 11
# speedup vs baseline: 1.0936x; 1.0715x over previous
"""Trainium2 Bass kernel for nn_Attention (Bahdanau-style additive attention).

Computation (reference):
    enc = encoder_outputs.transpose(1, 0, 2)            # [B, S, 2H]
    e_proj = enc @ w_e.T                                # [B, S, H]
    energy = tanh(h_proj[:, None, :] + e_proj + b)      # [B, S, H]
    att = energy @ v_w                                  # [B, S]
    out = softmax(att, axis=1)

Strategy: data-parallel over batch, 4 batch rows per core on 8 cores.
Per core, the big matmul (S x 2H) @ (2H x H) runs in bf16 on the PE:
  - encoder slice is DMA-transposed (xbar) from DRAM bf16 [S, 2H] into
    SBUF [128, 16, SG] so the contraction dim (e) lands on partitions;
    all 4 s-groups of a batch row stay resident so each weight chunk
    (the PE stationary operand) is reused for 4 matmuls and LDWEIGHTS
    stays hidden
  - psum[h_chunk(128), s(512)] accumulates over 16 e-chunks
  - ACT applies tanh with the per-partition bias c_b = h_proj + attn_b
    (h_proj is tiny: computed on host in fp32)
  - the v-dot reduction over h runs as an M=1 matmul accumulated over
    8 h-chunks; the 4 s-groups of a batch row share one PSUM bank at
    partitions {0,32,64,96} via tile_position
All DMAs (including xbar transposes) execute serially in emission order
(Tile serializes DMATranspose vs DMACopy transitions), so the weight
load is split by h-slice and only slice 0 gates the first matmuls; the
rest stream in behind batch row 0's transposes.
Softmax over S (tiny, [32, 2048]) runs on host in fp32.
"""

import sys

try:
    import concourse.bass as bass  # noqa: F401
except ImportError:
    sys.path.insert(0, "/opt/trn_rl_repo")

import numpy as np
import ml_dtypes

import concourse.bacc as bacc
import concourse.mybir as mybir
import concourse.tile as tile
from concourse.bass_utils import run_bass_kernel_spmd

HID = 1024
BATCH = 32
SRC_LEN = 2048

N_CORES = 8
B_LOC = BATCH // N_CORES      # 4
E = 2 * HID                   # 2048
SG = 512                      # matmul moving free dim (s per group)
N_SG = SRC_LEN // SG          # 4
N_EC = E // 128               # 16 e-chunks
N_HC = HID // 128             # 8 h-chunks

f32 = mybir.dt.float32
bf16 = mybir.dt.bfloat16

_NC_CACHE = {}


def _build():
    nc = bacc.Bacc(
        "TRN2", target_bir_lowering=False, debug=False, num_devices=N_CORES
    )
    enc = nc.declare_dram_parameter("enc", [B_LOC, SRC_LEN, E], bf16, isOutput=False)
    wT = nc.declare_dram_parameter("wT", [N_HC, 128, N_EC * 128], bf16, isOutput=False)
    cb = nc.declare_dram_parameter("cb", [128, B_LOC * N_HC], f32, isOutput=False)
    vT = nc.declare_dram_parameter("vT", [128, N_HC], bf16, isOutput=False)
    att = nc.declare_dram_parameter("att", [N_SG, B_LOC, SG], f32, isOutput=True)

    with tile.TileContext(nc) as tc:
        with (
            tc.tile_pool(name="const", bufs=1) as const_pool,
            tc.tile_pool(name="encT", bufs=2 * N_SG) as encT_pool,
            tc.tile_pool(name="energy", bufs=8) as en_pool,
            tc.tile_pool(name="attsb", bufs=1) as att_pool,
            tc.tile_pool(name="psum", bufs=7, space="PSUM") as psum_pool,
            tc.tile_pool(name="attps", bufs=1, space="PSUM") as attps_pool,
        ):
            # weights stored h-slice-major: w_sb[:, hc, c, :] is the [128,128]
            # stationary for (e-chunk c, h-chunk hc); the host pre-lays-out wT
            # as [hc][p][c*128+h'] so each h-slice is one fully-contiguous DMA
            w_sb = const_pool.tile([128, N_HC, N_EC, 128], bf16)
            cb_sb = const_pool.tile([128, B_LOC * N_HC], f32)
            vT_sb = const_pool.tile([128, N_HC], bf16)
            att_all = att_pool.tile([128, B_LOC * SG], f32)

            def load_w_slice(hc):
                nc.sync.dma_start(
                    w_sb[:, hc].rearrange("p c h -> p (c h)"),
                    wT[hc],
                )

            # h-slices 0-2 of the weights + the small consts: enough that the
            # PE can run 3 h-chunks per encoder tile while transposes stream
            for hc in range(3):
                load_w_slice(hc)
            nc.sync.dma_start(cb_sb[:], cb[:])
            nc.sync.dma_start(vT_sb[:], vT[:])

            # warmup tanh: pulls the ACT LUT-table-load dependency off the
            # first real tanh so no instruction needs multiple sync waits
            warm = const_pool.tile([128, 1], f32)
            nc.scalar.activation(
                warm[:], cb_sb[:, 0:1], mybir.ActivationFunctionType.Tanh
            )

            pending = []      # (att_ps, energy, hc, sg)
            done_banks = []   # (att_ps, b) fully accumulated

            def emit_vdots():
                while pending:
                    p_att, p_en, p_hc, p_sg = pending.pop(0)
                    nc.tensor.matmul(
                        p_att[32 * p_sg:32 * p_sg + 1, :],
                        lhsT=vT_sb[:, p_hc:p_hc + 1],
                        rhs=p_en[:],
                        start=(p_hc == 0),
                        stop=(p_hc == N_HC - 1),
                        tile_position=(0, 32 * p_sg),
                    )

            def drain_banks():
                while done_banks:
                    d_ps, d_b = done_banks.pop(0)
                    nc.vector.tensor_copy(
                        att_all[:, d_b * SG:(d_b + 1) * SG], d_ps[:]
                    )

            HALF = N_EC // 2

            def transpose_group(b, sg, split):
                encT = encT_pool.tile(
                    [128, N_EC, SG], bf16, tag="encT", name=f"encT_{b}_{sg}"
                )
                n_parts = 2 if split else 1
                step = N_EC // n_parts
                for part in range(n_parts):
                    nc.sync.dma_start(
                        out=encT[:, part * step:(part + 1) * step, :],
                        in_=enc[
                            b,
                            sg * SG:(sg + 1) * SG,
                            part * step * 128:(part + 1) * step * 128,
                        ],
                        transpose=True,
                    )
                return encT

            def tanh_energy(pss, b, hc, att_ps):
                for sg in range(N_SG):
                    energy = en_pool.tile(
                        [128, SG], bf16, tag="en", name=f"en_{b}_{hc}_{sg}"
                    )
                    nc.scalar.activation(
                        energy[:], pss[sg][:],
                        mybir.ActivationFunctionType.Tanh,
                        bias=cb_sb[:, b * N_HC + hc:b * N_HC + hc + 1],
                        scale=1.0,
                    )
                    pending.append((att_ps, energy, hc, sg))

            N_WARM_HC = 3  # h-chunks runnable per tile during the startup ramp

            for b in range(B_LOC):
                encTs = [
                    transpose_group(b, sg, split=(b == 0)) for sg in range(N_SG)
                ]
                if b == 0:
                    # remaining weight slices stream in behind the transposes
                    for hc in range(N_WARM_HC, N_HC):
                        load_w_slice(hc)
                att_ps = attps_pool.tile(
                    [128, SG], f32, tag="attps", name=f"attps_{b}"
                )
                hc_start = 0
                if b == 0:
                    # startup ramp: consume each s-group tile as its transpose
                    # lands, running h-chunks 0-2 per tile (their weight
                    # slices are the only ones loaded yet)
                    hc_start = N_WARM_HC
                    for sg in range(N_SG):
                        for hc in range(N_WARM_HC):
                            ps = psum_pool.tile(
                                [128, SG], f32, tag="ps", name=f"ps0_{hc}_{sg}"
                            )
                            for c in range(N_EC):
                                nc.tensor.matmul(
                                    ps[:],
                                    lhsT=w_sb[:, hc, c, :],
                                    rhs=encTs[sg][:, c, :],
                                    start=(c == 0),
                                    stop=(c == N_EC - 1),
                                )
                                if hc == 0 and c == 1:
                                    emit_vdots()
                            energy = en_pool.tile(
                                [128, SG], bf16, tag="en", name=f"en0_{hc}_{sg}"
                            )
                            nc.scalar.activation(
                                energy[:], ps[:],
                                mybir.ActivationFunctionType.Tanh,
                                bias=cb_sb[:, 0 * N_HC + hc:0 * N_HC + hc + 1],
                                scale=1.0,
                            )
                            pending.append((att_ps, energy, hc, sg))
                for hc in range(hc_start, N_HC):
                    pss = [
                        psum_pool.tile([128, SG], f32, tag="ps", name=f"ps_{b}_{hc}_{i}")
                        for i in range(N_SG)
                    ]
                    for c in range(N_EC):
                        for sg in range(N_SG):
                            nc.tensor.matmul(
                                pss[sg][:],
                                lhsT=w_sb[:, hc, c, :],
                                rhs=encTs[sg][:, c, :],
                                start=(c == 0),
                                stop=(c == N_EC - 1),
                            )
                        if c == 1:
                            emit_vdots()
                            drain_banks()
                    tanh_energy(pss, b, hc, att_ps)
                done_banks.append((att_ps, b))
            emit_vdots()
            drain_banks()
            nc.sync.dma_start(
                att[:],
                att_all.rearrange("(q t) (b s) -> q t b s", t=32, b=B_LOC)[:, 0],
            )
    nc.compile()
    return nc


def _get_nc():
    if "nc" not in _NC_CACHE:
        _NC_CACHE["nc"] = _build()
    return _NC_CACHE["nc"]


def kernel(hidden, encoder_outputs, attn_w, attn_b, v_w, _trace=False):
    hidden = np.asarray(hidden, dtype=np.float32)
    encoder_outputs = np.asarray(encoder_outputs, dtype=np.float32)
    attn_w = np.asarray(attn_w, dtype=np.float32)
    attn_b = np.asarray(attn_b, dtype=np.float32)
    v_w = np.asarray(v_w, dtype=np.float32)

    # host-side prologue (tiny): h_proj + bias
    c_b = hidden @ attn_w[:, :HID].T + attn_b          # [B, H] fp32
    w_e = attn_w[:, HID:]                                  # [H, E]
    wT_bf = np.ascontiguousarray(
        w_e.reshape(N_HC, 128, N_EC, 128).transpose(0, 3, 2, 1)
        .reshape(N_HC, 128, N_EC * 128)
    ).astype(ml_dtypes.bfloat16)
    vT_dev = np.ascontiguousarray(
        v_w.reshape(N_HC, 128).T
    ).astype(ml_dtypes.bfloat16)

    nc = _get_nc()
    in_maps = []
    for core in range(N_CORES):
        b0 = core * B_LOC
        enc_bf = np.ascontiguousarray(
            encoder_outputs[:, b0:b0 + B_LOC, :].transpose(1, 0, 2)
        ).astype(ml_dtypes.bfloat16)
        cb_dev = np.ascontiguousarray(
            c_b[b0:b0 + B_LOC]
            .reshape(B_LOC, N_HC, 128)
            .transpose(2, 0, 1)
            .reshape(128, B_LOC * N_HC)
        )
        in_maps.append({"enc": enc_bf, "wT": wT_bf, "cb": cb_dev, "vT": vT_dev})

    res = run_bass_kernel_spmd(
        nc, in_maps, core_ids=list(range(N_CORES)), trace=_trace
    )
    if _trace:
        _NC_CACHE["last_result"] = res

    att = np.concatenate(
        [
            res.results[c]["att"].transpose(1, 0, 2).reshape(B_LOC, SRC_LEN)
            for c in range(N_CORES)
        ],
        axis=0,
    )  # [B, S] logits, fp32

    # host softmax over S
    m = att.max(axis=1, keepdims=True)
    e = np.exp(att - m)
    out = e / e.sum(axis=1, keepdims=True)
    return out.astype(np.float32)
